# revision 4
# baseline (speedup 1.0000x reference)
"""Trainium2 Bass kernel for nn_MemoryModule (attention read over a memory bank).

reference:  logits = x @ mem^T ; attn = softmax(logits, axis=1) ; out = attn @ mem
shapes:     x [32768, 128], mem [4096, 128] -> out [32768, 128]

Sharding: data-parallel over batch across 8 cores (4096 rows each), memory
replicated.  No collectives needed (forward only).

Per-core algorithm (B=4096 local rows, M=4096, D=128), ACT-bound (~131us of
exp).  Layout: lt [m%128, chunk, b] per group of NB=512 batch columns.

  - PSUM: pA (4 banks) + pB (3 banks) alternate as lt tiles -> ACT ops of
    N=2048/1536 (9 per group vs 11 at 3+3), amortizing the ~352-cycle
    ACTIVATE overhead; pout (1 bank) accumulates mm2's outT [d, b].
  - Group 0 runs 3-wide ops so each A-tile's 4th bank is transpose
    scratch: memT staging batches (PE transpose + DVE f32r copy) ride
    there, two per window; groups 1-6 stage XT one group ahead in the
    pB slot right after its op-8 read.  A burst of tiny junk matmuls at
    the start warms the HAM clock gate (PE transposes alone don't).
  - One bf16 pt tile per group (fewer tile-reuse semaphores on the ACT
    queue); DVE accumulates pt into acc4 for the softmax denominator.
  - mm2 runs from a FIFO; a group's own chunks are gated until its op 3
    so the outT handoff (u16 copy) never stalls the in-order PE queue.
  - Finalize is split so no engine queue ever blocks on the gpsimd
    all-reduce latency: at group end: acc4 -> accf (DVE), gpsimd
    partition_all_reduce -> sums, XBAR-transpose -> snat (site S).
    When group g's last mm2 pops (mid g+1): u16 copy + XBAR-transpose
    -> unat (site A).  When g+1's last mm2 pops: reciprocal (tiny DVE),
    4 fused normalize+cast muls on the idle gpsimd, DMA out (site B).
  - The last group instead computes column sums with 4 ones-matmuls on
    the idle PE, normalizes outT in PSUM with a replicated reciprocal,
    and runs its epilogue in two 256-column slices to cut the drain tail.
"""

import numpy as np

import concourse.mybir as mybir
import concourse.tile as tile
from concourse import bacc
from concourse.bass_isa import ReduceOp
from concourse.masks import make_identity

B, M, D = 32768, 4096, 128
NCORES = 8
BLOC = B // NCORES  # 4096 rows per core
P = 128
NB = 512            # batch columns per group
NG = BLOC // NB     # 8 groups
MCHUNKS = M // P    # 32

F32 = mybir.dt.float32
F32R = mybir.dt.float32r
BF16 = mybir.dt.bfloat16
EXP = mybir.ActivationFunctionType.Exp

# per-group ACT-op chunk widths (even index -> pA tile, odd -> pB tile)
PAT_G0 = [3, 3, 3, 3, 3, 3, 3, 3, 3, 3, 2]   # A-tiles keep bank 3 as scratch
PAT = [4, 3, 4, 3, 4, 3, 4, 3, 4]


def build_nc():
    nc = bacc.Bacc(
        "TRN2", target_bir_lowering=False, debug=False, enable_asserts=False
    )
    x = nc.dram_tensor("x", [BLOC, D], F32, kind="ExternalInput").ap()
    mem = nc.dram_tensor("mem", [M, D], F32, kind="ExternalInput").ap()
    out = nc.dram_tensor("out", [BLOC, D], F32, kind="ExternalOutput").ap()

    with tile.TileContext(nc) as tc:
        with (
            tc.tile_pool(name="const", bufs=1) as constp,
            tc.tile_pool(name="pt", bufs=2) as ptp,
            tc.tile_pool(name="acc", bufs=2) as accp,
            tc.tile_pool(name="fin", bufs=2) as finp,
            tc.tile_pool(name="pA", bufs=1, space="PSUM") as pA,
            tc.tile_pool(name="pB", bufs=1, space="PSUM") as pB,
            tc.tile_pool(name="pout", bufs=1, space="PSUM") as pout,
        ):
            ident = constp.tile([P, P], F32)
            make_identity(nc, ident)
            expbias = constp.tile([P, 1], F32)
            nc.vector.memset(expbias, -45.0)
            ones_bf = constp.tile([P, 1], BF16)
            nc.vector.memset(ones_bf, 1.0)
            ones128 = constp.tile([P, P], BF16)
            nc.vector.memset(ones128, 1.0)

            # Natural-layout staging: partition = row%128, free = (chunk, d).
            stage_m = constp.tile([P, MCHUNKS, D], F32)
            mem_t = mem.rearrange("(c p) d -> p c d", p=P)
            stage_x = constp.tile([P, MCHUNKS, D], F32)
            x_t = x.rearrange("(c p) d -> p c d", p=P)
            for dst, src, s in (
                (stage_m, mem_t, slice(0, 4)),
                (stage_x, x_t, slice(0, 4)),
                (stage_m, mem_t, slice(4, 8)),
                (stage_x, x_t, slice(4, 8)),
                (stage_m, mem_t, slice(8, 16)),
                (stage_x, x_t, slice(8, 16)),
                (stage_m, mem_t, slice(16, 32)),
                (stage_x, x_t, slice(16, 32)),
            ):
                nc.sync.dma_start(out=dst[:, s, :], in_=src[:, s, :])

            memT = constp.tile([P, M], F32R)
            XT = constp.tile([P, BLOC], F32R)
            mem_nat = constp.tile([P, MCHUNKS, D], BF16)
            for q in range(MCHUNKS // 4):
                s = slice(4 * q, 4 * q + 4)
                nc.gpsimd.tensor_copy(out=mem_nat[:, s, :], in_=stage_m[:, s, :])

            def stage4(src, dst, q, scratch):
                """Transpose 4 chunks of src into scratch (psum [P, NB] view as
                4x[P,P]), then DVE-copy (f32->f32r) to dst[:, q*512:(q+1)*512]."""
                for j in range(4):
                    nc.tensor.transpose(
                        scratch[:, j * P : (j + 1) * P], src[:, 4 * q + j, :], ident
                    )
                nc.vector.tensor_copy(
                    out=dst[:, q * 4 * P : (q + 1) * 4 * P], in_=scratch
                )

            # HAM warm-up: dense stream of tiny matmuls so the clock gate
            # opens before the staging transposes + mm1 begin.
            junk = pB.tile([P, 3, NB], F32, tag="lt", name="junk")
            for j in range(24):
                nc.tensor.matmul(
                    junk[0:1, 0, j : j + 1],
                    ones_bf,
                    ones_bf,
                    start=True,
                    stop=True,
                    skip_group_check=True,
                )

            # prelude staging: m q0, q1 and x g0 in the free pA/pB slots.
            pre = pA.tile([P, 4, NB], F32, tag="lt", name="pre")
            preB = pB.tile([P, 3, NB], F32, tag="lt", name="preB")
            stage4(stage_m, memT, 0, pre[:, 0, :])
            stage4(stage_x, XT, 0, preB[:, 0, :])
            stage4(stage_m, memT, 1, pre[:, 1, :])

            # g0 A-op scratch windows: two staging batches each.
            g0_windows = {
                0: [("m", 2), ("m", 3)],
                2: [("m", 4), ("m", 5)],
                4: [("m", 6), ("m", 7)],
                6: [("x", 1)],
                8: [("x", 2)],
            }

            mm2q = []      # FIFO of (pt_tile, mc, outT, g)
            fin_a = {}     # g -> outT          (set at group end)
            fin_s = {}     # g -> snat          (set at group end, g<7)
            fin_b = {}     # g -> (unat, outT)  (set at site A)
            st = {"g": 0, "t": 0}

            def site_a(g, outT):
                """g's last mm2 just issued: free outT via bf16 copy, transpose."""
                u16 = finp.tile([P, NB], BF16, tag="u16", name=f"u16_{g}")
                nc.vector.tensor_copy(out=u16, in_=outT)
                unat = finp.tile([P, 4, P], BF16, tag="unat", name=f"unat_{g}")
                nc.sync.dma_start_transpose(out=unat, in_=u16)
                fin_b[g] = unat

            def site_b(g):
                """Normalize+cast+store group g (snat/unat long since ready)."""
                unat = fin_b.pop(g)
                snat = fin_s.pop(g)
                rs4 = finp.tile([P, 4], F32, tag="rs4", name=f"rs4_{g}")
                nc.vector.reciprocal(rs4, snat[:, :, 0])
                outf = finp.tile([P, 4, P], F32, tag="outf", name=f"outf_{g}")
                for j in range(4):
                    nc.gpsimd.tensor_scalar_mul(
                        outf[:, j, :], unat[:, j, :], rs4[:, j : j + 1]
                    )
                nc.sync.dma_start(
                    out=out[g * NB : (g + 1) * NB, :].rearrange(
                        "(j p) d -> p j d", p=P
                    ),
                    in_=outf,
                )

            def issue_mm2(budget):
                issued = 0
                while mm2q and issued < budget:
                    qpt, qmc, qoutT, qg = mm2q[0]
                    # gate: a group's own chunks wait until its op 3, so the
                    # outT handoff never stalls the in-order PE queue
                    if qg == st["g"] and st["t"] < 2:
                        break
                    mm2q.pop(0)
                    nc.tensor.matmul(
                        qoutT,
                        mem_nat[:, qmc, :],
                        qpt[:, qmc, :],
                        start=(qmc == 0),
                        stop=(qmc == MCHUNKS - 1),
                        skip_group_check=True,
                    )
                    issued += 1
                    if qmc == MCHUNKS - 1 and qg < NG - 1:
                        site_a(qg, qoutT)
                        if qg >= 1:
                            site_b(qg - 1)

            for g in range(NG):
                pat = PAT_G0 if g == 0 else PAT
                st["g"] = g
                xtg = XT[:, g * NB : (g + 1) * NB]
                outT = pout.tile([P, NB], F32, tag="pb", name=f"outT_{g}")
                acc4 = accp.tile([P, 4, NB], BF16, tag="acc", name=f"acc_{g}")
                pt = ptp.tile([P, MCHUNKS, NB], BF16, tag="pt", name=f"pt_{g}")
                mc0 = 0
                for t, w in enumerate(pat):
                    st["t"] = t
                    if t % 2 == 0:
                        lt = pA.tile([P, 4, NB], F32, tag="lt", name=f"ltA_{g}_{t}")
                    else:
                        lt = pB.tile([P, 3, NB], F32, tag="lt", name=f"ltB_{g}_{t}")
                    for c in range(w):
                        mc = mc0 + c
                        nc.tensor.matmul(
                            lt[:, c, :],
                            memT[:, mc * P : (mc + 1) * P],
                            xtg,
                            start=True,
                            stop=True,
                        )
                    # staging rides after this op's mm1 stream
                    if g == 0 and t in g0_windows:
                        for kind, q in g0_windows[t]:
                            if kind == "m":
                                stage4(stage_m, memT, q, lt[:, 3, :])
                            else:
                                stage4(stage_x, XT, q, lt[:, 3, :])
                    if g >= 1 and t == 8 and g + 2 < NG:
                        # op9's mm1 was just emitted; pB's op8 tile frees
                        # when ACT op8 completes -> stage XT two groups out
                        # (g0's windows covered x1 and x2)
                        tp = pB.tile([P, 3, NB], F32, tag="lt", name=f"tpx_{g}")
                        stage4(stage_x, XT, g + 2, tp[:, 0, :])
                    nc.scalar.activation(
                        pt[:, mc0 : mc0 + w, :], lt[:, :w, :], EXP, bias=expbias
                    )
                    if t == 0:
                        nc.vector.tensor_copy(
                            out=acc4[:, :w, :], in_=pt[:, mc0 : mc0 + w, :]
                        )
                        if w < 4:
                            nc.vector.memset(acc4[:, w:, :], 0)
                    else:
                        nc.vector.tensor_add(
                            acc4[:, :w, :],
                            acc4[:, :w, :],
                            pt[:, mc0 : mc0 + w, :],
                        )
                    for c in range(w):
                        mm2q.append((pt, mc0 + c, outT, g))
                    issue_mm2(2 if g == 0 else 5)
                    mc0 += w

                fin_a[g] = outT
                if g < NG - 1:
                    # site S: column sums via gpsimd all-reduce + transpose
                    acc2 = finp.tile([P, 2, NB], BF16, tag="acc2", name=f"acc2_{g}")
                    nc.vector.tensor_add(acc2, acc4[:, 0:2, :], acc4[:, 2:4, :])
                    accf = finp.tile([P, NB], BF16, tag="accf", name=f"accf_{g}")
                    nc.vector.tensor_add(accf, acc2[:, 0, :], acc2[:, 1, :])
                    sums = finp.tile([P, NB], BF16, tag="sums", name=f"sums_{g}")
                    nc.gpsimd.partition_all_reduce(sums, accf, P, ReduceOp.add)
                    snat = finp.tile([P, 4, P], BF16, tag="snat", name=f"snat_{g}")
                    nc.sync.dma_start_transpose(out=snat, in_=sums)
                    fin_s[g] = snat
                else:
                    # tail: PE is free -> accumulate column sums in PSUM
                    sums_ps = pB.tile([P, NB], F32, tag="lt", name="sums_ps")
                    for c in range(4):
                        nc.tensor.matmul(
                            sums_ps,
                            ones128,
                            acc4[:, c, :],
                            start=(c == 0),
                            stop=(c == 3),
                            skip_group_check=True,
                        )
                    rec_rep = finp.tile([P, NB], F32, tag="rec", name="rec_7")
                    nc.vector.reciprocal(rec_rep, sums_ps)

            # drain the software pipeline (gate: st past all groups)
            st["g"] = NG
            while mm2q:
                issue_mm2(len(mm2q))
            site_b(NG - 2)

            # tail group: normalize in PSUM orientation (replicated recip),
            # sliced epilogue to cut the serial drain
            g = NG - 1
            outT = fin_a[g]
            for s in range(2):
                w = NB // 2
                cs = slice(s * w, (s + 1) * w)
                u16 = finp.tile([P, w], BF16, tag="u16t", name=f"u16t_{s}")
                nc.vector.tensor_mul(u16, outT[:, cs], rec_rep[:, cs])
                unat = finp.tile([P, 2, P], BF16, tag="unatt", name=f"unatt_{s}")
                nc.sync.dma_start_transpose(out=unat, in_=u16)
                outf = finp.tile([P, 2, P], F32, tag="outft", name=f"outft_{s}")
                nc.vector.tensor_copy(out=outf, in_=unat)
                nc.sync.dma_start(
                    out=out[g * NB + s * w : g * NB + (s + 1) * w, :].rearrange(
                        "(j p) d -> p j d", p=P
                    ),
                    in_=outf,
                )

    nc.compile()
    return nc


_NC_CACHE = None


def _get_nc():
    global _NC_CACHE
    if _NC_CACHE is None:
        _NC_CACHE = build_nc()
    return _NC_CACHE


def _in_maps(local_stats, memory):
    local_stats = np.ascontiguousarray(local_stats, dtype=np.float32)
    memory = np.ascontiguousarray(memory, dtype=np.float32)
    return [
        {
            "x": np.ascontiguousarray(local_stats[i * BLOC : (i + 1) * BLOC]),
            "mem": memory,
        }
        for i in range(NCORES)
    ]


def run_spmd(local_stats, memory, **kwargs):
    """Run on all 8 cores; returns BassKernelResults (for test harness use)."""
    from concourse.bass_utils import run_bass_kernel_spmd

    nc = _get_nc()
    return run_bass_kernel_spmd(
        nc, _in_maps(local_stats, memory), core_ids=list(range(NCORES)), **kwargs
    )


def kernel(local_stats, memory):
    res = run_spmd(local_stats, memory)
    return np.concatenate([r["out"] for r in res.results], axis=0)
